# revision 8
# baseline (speedup 1.0000x reference)
"""CCMLite kernel for Trainium2: GroupNorm(affine=False) + low-rank channel mix.

out = x_norm + u @ (v^T @ x_norm) + shift, with x_norm = groupnorm(x).

Sharding: data-parallel over batch B=16 across 8 cores (2 batch elems/core).
No collectives needed.

Device-side algebra (per batch element, all folded into PE matmuls):
  per-channel stats (bn_stats/bn_aggr on DVE)
  group stats via mask matmul  (PE)    -> group mean m_g, rstd s_g
  broadcast to channels via maskT matmul (PE)
  vs[c,r]  = v[c,r] * s_c                      (DVE, tiny)
  kvec[r]  = sum_c vs[c,r] * m_c               (PE, tiny)
  const_c  = shift_c - m_c*s_c - sum_r u[c,r]*kvec[r]
  vtx[r,n] = sum_c vs[c,r] * x[c,n]            (PE, K=128 x2)
  out[c,n] = s_c*x[c,n] + sum_r u[c,r]*vtx[r,n] + const_c
 where the last line is computed either as
   (a) PE: aug-matmul (K=13: rows u^T + const row against [vtx; ones])
       + diag(rstd) matmul accumulate, then ACT copies PSUM->SBUF, or
   (b) PE aug-matmul only, then DVE scalar_tensor_tensor:
       out = (x * s_c) + psum_mixed
 chunks alternate (a)/(b) to balance ACT and DVE load.
"""

from contextlib import ExitStack

import numpy as np

import concourse.bass as bass
import concourse.tile as tile
from concourse import bacc, mybir
from concourse.bass_utils import run_bass_kernel_spmd
from concourse.masks import make_identity

N_CORES = 8
B, C, H, W = 16, 256, 64, 64
HW = H * W            # 4096
R = 12                # low rank
G = 32                # groups
GPC = C // G          # 8 channels per group
P = 128               # partitions
CB = C // P           # 2 channel blocks
BPC = B // N_CORES    # 2 batch elements per core
NCK = HW // 512       # 8 chunks of 512
EPS = 1e-6
F32 = mybir.dt.float32

_MULT = mybir.AluOpType.mult
_ADD = mybir.AluOpType.add


def build_nc(stt_mod=2):
    """Build the per-core Bass program. stt_mod: chunk j uses the DVE
    scalar_tensor_tensor path when (j + cb) % stt_mod == 0, else the
    PE-diag + ACT-copy path. stt_mod=0 -> always PE path."""
    nc = bacc.Bacc(None, target_bir_lowering=False)
    x_d = nc.dram_tensor("x", [BPC, C, HW], F32, kind="ExternalInput")
    ut_d = nc.dram_tensor("ut", [BPC, CB, R, P], F32, kind="ExternalInput")
    v_d = nc.dram_tensor("v", [BPC, CB, P, R], F32, kind="ExternalInput")
    shift_d = nc.dram_tensor("shift", [BPC, CB, P, 1], F32, kind="ExternalInput")
    gmask_d = nc.dram_tensor("gmask", [P, 16], F32, kind="ExternalInput")
    gmaskT_d = nc.dram_tensor("gmaskT", [16, P], F32, kind="ExternalInput")
    out_d = nc.dram_tensor("out", [BPC, C, HW], F32, kind="ExternalOutput")

    with tile.TileContext(nc) as tc, ExitStack() as ctx:
        consts = ctx.enter_context(tc.tile_pool(name="consts", bufs=1))
        xp = ctx.enter_context(tc.tile_pool(name="xp", bufs=8))
        outp = ctx.enter_context(tc.tile_pool(name="outp", bufs=6))
        vaugp = ctx.enter_context(tc.tile_pool(name="vaugp", bufs=2))
        smalls = ctx.enter_context(tc.tile_pool(name="smalls", bufs=4))
        ps_small = ctx.enter_context(tc.tile_pool(name="ps_small", bufs=2, space="PSUM"))
        ps_vtx = ctx.enter_context(tc.tile_pool(name="ps_vtx", bufs=2, space="PSUM"))
        ps_out = ctx.enter_context(tc.tile_pool(name="ps_out", bufs=3, space="PSUM"))

        ident = consts.tile([P, P], F32)
        make_identity(nc, ident)
        gmask = consts.tile([P, 16], F32)
        nc.sync.dma_start(out=gmask, in_=gmask_d[:, :])
        gmaskT = consts.tile([16, P], F32)
        nc.sync.dma_start(out=gmaskT, in_=gmaskT_d[:, :])
        eps_t = consts.tile([16, 1], F32)
        nc.vector.memset(eps_t, EPS)

        for b in range(BPC):
            # ---- load x: 4 tiles of [128, 2048] per batch elem ----
            xt = {}
            for cb in range(CB):
                for h in range(2):
                    t = xp.tile([P, 2048], F32, tag="xt")
                    nc.sync.dma_start(
                        out=t,
                        in_=x_d[b, cb * P:(cb + 1) * P, h * 2048:(h + 1) * 2048],
                    )
                    xt[(cb, h)] = t

            # ---- per-channel stats ----
            mvs = []
            for cb in range(CB):
                st = smalls.tile([P, NCK, 6], F32, tag="bstats")
                for j in range(NCK):
                    nc.vector.bn_stats(
                        out=st[:, j:j + 1, :],
                        in_=xt[(cb, j // 4)][:, (j % 4) * 512:(j % 4 + 1) * 512],
                    )
                mv = smalls.tile([P, 2], F32, tag=f"mv{cb}")
                nc.vector.bn_aggr(out=mv, in_=st)
                mvs.append(mv)

            # chs cols: [mean_cb0, E2_cb0, mean_cb1, E2_cb1]
            chs = smalls.tile([P, 4], F32, tag="chs")
            for cb in range(CB):
                mv = mvs[cb]
                nc.vector.tensor_copy(out=chs[:, 2 * cb:2 * cb + 1], in_=mv[:, 0:1])
                nc.vector.tensor_mul(
                    out=chs[:, 2 * cb + 1:2 * cb + 2], in0=mv[:, 0:1], in1=mv[:, 0:1])
                nc.vector.tensor_add(
                    out=chs[:, 2 * cb + 1:2 * cb + 2],
                    in0=chs[:, 2 * cb + 1:2 * cb + 2], in1=mv[:, 1:2])

            # ---- group stats: gsum[g, :] = sum over 8 chans ----
            gsum = ps_small.tile([16, 4], F32, tag="ps")
            nc.tensor.matmul(gsum, lhsT=gmask, rhs=chs, start=True, stop=True)

            # gvals cols: [rstd_cb0, mean_cb0, rstd_cb1, mean_cb1]
            gvals = smalls.tile([16, 4], F32, tag="gvals")
            tmpg = smalls.tile([16, 4], F32, tag="tmpg")
            for cb in range(CB):
                gmean = gvals[:, 2 * cb + 1:2 * cb + 2]
                nc.vector.tensor_scalar_mul(
                    out=gmean, in0=gsum[:, 2 * cb:2 * cb + 1], scalar1=1.0 / GPC)
                gm2 = tmpg[:, 2 * cb:2 * cb + 1]
                nc.vector.tensor_scalar_mul(
                    out=gm2, in0=gsum[:, 2 * cb + 1:2 * cb + 2], scalar1=1.0 / GPC)
                gvar = tmpg[:, 2 * cb + 1:2 * cb + 2]
                nc.vector.tensor_mul(out=gvar, in0=gmean, in1=gmean)
                nc.vector.tensor_sub(out=gvar, in0=gm2, in1=gvar)
                # std = sqrt(var + eps)
                nc.scalar.activation(
                    out=gvar, in_=gvar,
                    func=mybir.ActivationFunctionType.Sqrt, bias=eps_t[:, 0:1],
                    scale=1.0)
                nc.vector.reciprocal(out=gvals[:, 2 * cb:2 * cb + 1], in_=gvar)

            # ---- broadcast to per-channel: sm cols [s0, m0, s1, m1] ----
            bc = ps_small.tile([P, 4], F32, tag="ps")
            nc.tensor.matmul(bc, lhsT=gmaskT, rhs=gvals, start=True, stop=True)
            sm = smalls.tile([P, 4], F32, tag="sm")
            nc.vector.tensor_copy(out=sm, in_=bc)

            # ---- per-cb small prep ----
            vss, diags, augs = [], [], []
            kvec = ps_small.tile([R, 1], F32, tag="ps")
            for cb in range(CB):
                s_ap = sm[:, 2 * cb:2 * cb + 1]
                m_ap = sm[:, 2 * cb + 1:2 * cb + 2]
                vt = smalls.tile([P, R], F32, tag=f"vt{cb}")
                nc.sync.dma_start(out=vt, in_=v_d[b, cb])
                vs = smalls.tile([P, R], F32, tag=f"vs{cb}")
                nc.vector.tensor_scalar_mul(out=vs, in0=vt, scalar1=s_ap)
                diag = smalls.tile([P, P], F32, tag=f"diag{cb}")
                nc.vector.tensor_scalar_mul(out=diag, in0=ident, scalar1=s_ap)
                nc.tensor.matmul(
                    kvec, lhsT=vs, rhs=m_ap, start=(cb == 0), stop=(cb == CB - 1))
                vss.append(vs)
                diags.append(diag)
            kvs = smalls.tile([R, 1], F32, tag="kvs")
            nc.vector.tensor_copy(out=kvs, in_=kvec)

            for cb in range(CB):
                s_ap = sm[:, 2 * cb:2 * cb + 1]
                m_ap = sm[:, 2 * cb + 1:2 * cb + 2]
                aug = smalls.tile([R + 1, P], F32, tag=f"aug{cb}")
                nc.sync.dma_start(out=aug[0:R, :], in_=ut_d[b, cb])
                ukv = ps_small.tile([P, 1], F32, tag="ps")
                nc.tensor.matmul(ukv, lhsT=aug[0:R, :], rhs=kvs, start=True, stop=True)
                shf = smalls.tile([P, 1], F32, tag=f"shf{cb}")
                nc.sync.dma_start(out=shf, in_=shift_d[b, cb])
                cst = smalls.tile([P, 1], F32, tag=f"cst{cb}")
                nc.vector.tensor_mul(out=cst, in0=m_ap, in1=s_ap)
                nc.vector.tensor_sub(out=cst, in0=shf, in1=cst)
                nc.vector.tensor_sub(out=cst, in0=cst, in1=ukv)
                ctp = ps_small.tile([1, P], F32, tag="ps")
                nc.tensor.transpose(out=ctp, in_=cst, identity=ident)
                cstrow = smalls.tile([1, P], F32, tag=f"cstrow{cb}")
                nc.scalar.copy(out=cstrow, in_=ctp)
                # compute engines can't write at start partition 12; DMA can
                nc.sync.dma_start(out=aug[R:R + 1, :], in_=cstrow)
                augs.append(aug)

            # ---- stage A: vtx[r, n] ----
            vaug = vaugp.tile([R + 1, HW], F32, tag="vaug")
            # whole-tile memset (start partition must be 0/32/64/96);
            # rows 0..R-1 are overwritten by the vtx copies below
            nc.gpsimd.memset(vaug, 1.0)
            for j in range(NCK):
                vtxp = ps_vtx.tile([R, 512], F32, tag="vtx")
                for cb in range(CB):
                    nc.tensor.matmul(
                        vtxp,
                        lhsT=vss[cb],
                        rhs=xt[(cb, j // 4)][:, (j % 4) * 512:(j % 4 + 1) * 512],
                        start=(cb == 0), stop=(cb == CB - 1))
                nc.scalar.copy(out=vaug[0:R, j * 512:(j + 1) * 512], in_=vtxp)

            # ---- stage B: out = s*x + u@vtx + const ----
            for cb in range(CB):
                for h in range(2):
                    osb = outp.tile([P, 2048], F32, tag="osb")
                    for q in range(4):
                        j = h * 4 + q
                        rhs_x = xt[(cb, h)][:, q * 512:(q + 1) * 512]
                        o_ap = osb[:, q * 512:(q + 1) * 512]
                        pm = ps_out.tile([P, 512], F32, tag="pout")
                        use_stt = stt_mod > 0 and (j + cb) % stt_mod == 0
                        nc.tensor.matmul(
                            pm, lhsT=augs[cb],
                            rhs=vaug[:, j * 512:(j + 1) * 512],
                            start=True, stop=use_stt)
                        if use_stt:
                            nc.vector.scalar_tensor_tensor(
                                out=o_ap, in0=rhs_x,
                                scalar=sm[:, 2 * cb:2 * cb + 1], in1=pm,
                                op0=_MULT, op1=_ADD)
                        else:
                            nc.tensor.matmul(
                                pm, lhsT=diags[cb], rhs=rhs_x,
                                start=False, stop=True)
                            nc.scalar.copy(out=o_ap, in_=pm)
                    nc.sync.dma_start(
                        out=out_d[b, cb * P:(cb + 1) * P, h * 2048:(h + 1) * 2048],
                        in_=osb)
    # Bacc defers register allocation to its compile()/finalize() pipeline
    nc.finalize()
    return nc


def _host_prep(x, ccm_params):
    x = np.ascontiguousarray(np.asarray(x, dtype=np.float32).reshape(B, C, HW))
    cp = np.asarray(ccm_params, dtype=np.float32)
    u = cp[:, :C * R].reshape(B, C, R)
    v = cp[:, C * R:2 * C * R].reshape(B, C, R)
    shift = cp[:, 2 * C * R:].reshape(B, C)
    ut = np.ascontiguousarray(u.reshape(B, CB, P, R).transpose(0, 1, 3, 2))
    vv = np.ascontiguousarray(v.reshape(B, CB, P, R))
    sh = np.ascontiguousarray(shift.reshape(B, CB, P, 1))
    gmask = np.zeros((P, 16), np.float32)
    gmask[np.arange(P), np.arange(P) // GPC] = 1.0
    gmaskT = np.ascontiguousarray(gmask.T)
    in_maps = []
    for c in range(N_CORES):
        bs = slice(c * BPC, (c + 1) * BPC)
        in_maps.append({
            "x": x[bs], "ut": ut[bs], "v": vv[bs], "shift": sh[bs],
            "gmask": gmask, "gmaskT": gmaskT,
        })
    return in_maps


def kernel(x, ccm_params, _trace=False, _stt_mod=2):
    in_maps = _host_prep(x, ccm_params)
    nc = build_nc(stt_mod=_stt_mod)
    res = run_bass_kernel_spmd(
        nc, in_maps, core_ids=list(range(N_CORES)), trace=_trace)
    out = np.concatenate([r["out"] for r in res.results], axis=0)
    out = out.reshape(B, C, H, W)
    if _trace:
        return out, res
    return out


# revision 10
# speedup vs baseline: 1.2707x; 1.2707x over previous
"""CCMLite kernel for Trainium2: GroupNorm(affine=False) + low-rank channel mix.

out = x_norm + u @ (v^T @ x_norm) + shift, with x_norm = groupnorm(x).

Sharding: data-parallel over batch B=16 across 8 cores (2 batch elems/core).
No collectives needed.

Device-side algebra (per batch element):
  per-channel stats (bn_stats/bn_aggr on DVE, f32)
  group stats via mask matmul (PE)       -> group mean m_g, rstd s_g
  broadcast to channels via maskT matmul (PE)
  vs[c,r]  = v[c,r] * s_c
  kvec[r]  = sum_c vs[c,r] * m_c
  const_c  = shift_c - m_c*s_c - sum_r u[c,r]*kvec[r]
  vtx[r,n] = sum_c vs[c,r] * x[c,n]      (PE, bf16, K=128 x2)
  out[c,n] = s_c*x[c,n] + sum_r u[c,r]*vtx[r,n] + const_c
 where the last line is computed either as
   (a) PE: aug-matmul (K=13: rows u^T + const row against [vtx; ones])
       + diag(rstd) matmul accumulate (bf16), then ACT copies PSUM->SBUF, or
   (b) PE aug-matmul only, then DVE scalar_tensor_tensor (f32):
       out = (x * s_c) + psum_mixed
 chunks alternate (a)/(b) to balance ACT and DVE load.

The heavy matmuls run in bf16 (fp32 matmuls cost 2 PE passes); x is cast
f32->bf16 once on the otherwise-idle GpSimd engine. Stats and the final
accumulation stay f32 (PSUM is always f32).
"""

from contextlib import ExitStack

import numpy as np

import concourse.bass as bass
import concourse.tile as tile
from concourse import bacc, mybir
from concourse.bass_utils import run_bass_kernel_spmd
from concourse.masks import make_identity

N_CORES = 8
B, C, H, W = 16, 256, 64, 64
HW = H * W            # 4096
R = 12                # low rank
G = 32                # groups
GPC = C // G          # 8 channels per group
P = 128               # partitions
CB = C // P           # 2 channel blocks
BPC = B // N_CORES    # 2 batch elements per core
NCK = HW // 512       # 8 chunks of 512
EPS = 1e-6
F32 = mybir.dt.float32
BF16 = mybir.dt.float16  # fp16: same PE speed as bf16, 8x better mantissa

_MULT = mybir.AluOpType.mult
_ADD = mybir.AluOpType.add


def build_nc(stt_mod=2):
    """Build the per-core Bass program. stt_mod: chunk j uses the DVE
    scalar_tensor_tensor path when (j + cb) % stt_mod == 0, else the
    PE-diag + ACT-copy path. stt_mod=0 -> always PE path."""
    nc = bacc.Bacc(None, target_bir_lowering=False)
    x_d = nc.dram_tensor("x", [BPC, C, HW], F32, kind="ExternalInput")
    ut_d = nc.dram_tensor("ut", [BPC, CB, R, P], BF16, kind="ExternalInput")
    v_d = nc.dram_tensor("v", [BPC, CB, P, R], F32, kind="ExternalInput")
    shift_d = nc.dram_tensor("shift", [BPC, CB, P, 1], F32, kind="ExternalInput")
    gmask_d = nc.dram_tensor("gmask", [P, 16], F32, kind="ExternalInput")
    gmaskT_d = nc.dram_tensor("gmaskT", [16, P], F32, kind="ExternalInput")
    out_d = nc.dram_tensor("out", [BPC, C, HW], F32, kind="ExternalOutput")

    with tile.TileContext(nc) as tc, ExitStack() as ctx:
        consts = ctx.enter_context(tc.tile_pool(name="consts", bufs=1))
        xp = ctx.enter_context(tc.tile_pool(name="xp", bufs=8))
        xbp = ctx.enter_context(tc.tile_pool(name="xbp", bufs=8))
        outp = ctx.enter_context(tc.tile_pool(name="outp", bufs=6))
        smalls = ctx.enter_context(tc.tile_pool(name="smalls", bufs=4))
        ps_small = ctx.enter_context(tc.tile_pool(name="ps_small", bufs=2, space="PSUM"))
        ps_vtx = ctx.enter_context(tc.tile_pool(name="ps_vtx", bufs=2, space="PSUM"))
        ps_out = ctx.enter_context(tc.tile_pool(name="ps_out", bufs=3, space="PSUM"))

        ident = consts.tile([P, P], F32)
        make_identity(nc, ident)
        ident_bf = consts.tile([P, P], BF16)
        make_identity(nc, ident_bf)
        gmask = consts.tile([P, 16], F32)
        nc.sync.dma_start(out=gmask, in_=gmask_d[:, :])
        gmaskT = consts.tile([16, P], F32)
        nc.sync.dma_start(out=gmaskT, in_=gmaskT_d[:, :])
        eps_t = consts.tile([16, 1], F32)
        nc.vector.memset(eps_t, EPS)
        # two persistent vtx-aug rhs tiles (rows 0..R-1 rewritten per batch,
        # row R stays 1.0); hoisting the memset off the per-batch path
        vaugs = []
        for i in range(2):
            va = consts.tile([R + 1, HW], BF16, tag=f"vaug{i}")
            nc.gpsimd.memset(va, 1.0)
            vaugs.append(va)

        for b in range(BPC):
            # ---- load x (f32) and cast to bf16 on GpSimd ----
            xt, xbt = {}, {}
            for cb in range(CB):
                for h in range(2):
                    t = xp.tile([P, 2048], F32, tag="xt")
                    nc.sync.dma_start(
                        out=t,
                        in_=x_d[b, cb * P:(cb + 1) * P, h * 2048:(h + 1) * 2048],
                    )
                    xt[(cb, h)] = t
                    tb = xbp.tile([P, 2048], BF16, tag="xbt")
                    nc.gpsimd.tensor_copy(out=tb, in_=t)
                    xbt[(cb, h)] = tb

            # ---- per-channel stats (f32) ----
            mvs = []
            for cb in range(CB):
                st = smalls.tile([P, NCK, 6], F32, tag="bstats")
                for j in range(NCK):
                    nc.vector.bn_stats(
                        out=st[:, j:j + 1, :],
                        in_=xt[(cb, j // 4)][:, (j % 4) * 512:(j % 4 + 1) * 512],
                    )
                mv = smalls.tile([P, 2], F32, tag=f"mv{cb}")
                nc.vector.bn_aggr(out=mv, in_=st)
                mvs.append(mv)

            # chs cols: [mean_cb0, E2_cb0, mean_cb1, E2_cb1]
            chs = smalls.tile([P, 4], F32, tag="chs")
            for cb in range(CB):
                mv = mvs[cb]
                nc.vector.tensor_copy(out=chs[:, 2 * cb:2 * cb + 1], in_=mv[:, 0:1])
                nc.vector.tensor_mul(
                    out=chs[:, 2 * cb + 1:2 * cb + 2], in0=mv[:, 0:1], in1=mv[:, 0:1])
                nc.vector.tensor_add(
                    out=chs[:, 2 * cb + 1:2 * cb + 2],
                    in0=chs[:, 2 * cb + 1:2 * cb + 2], in1=mv[:, 1:2])

            # ---- group stats: gsum[g, :] = sum over 8 chans ----
            gsum = ps_small.tile([16, 4], F32, tag="ps")
            nc.tensor.matmul(gsum, lhsT=gmask, rhs=chs, start=True, stop=True)

            # gvals cols: [rstd_cb0, mean_cb0, rstd_cb1, mean_cb1]
            gvals = smalls.tile([16, 4], F32, tag="gvals")
            tmpg = smalls.tile([16, 4], F32, tag="tmpg")
            for cb in range(CB):
                gmean = gvals[:, 2 * cb + 1:2 * cb + 2]
                nc.vector.tensor_scalar_mul(
                    out=gmean, in0=gsum[:, 2 * cb:2 * cb + 1], scalar1=1.0 / GPC)
                gm2 = tmpg[:, 2 * cb:2 * cb + 1]
                nc.vector.tensor_scalar_mul(
                    out=gm2, in0=gsum[:, 2 * cb + 1:2 * cb + 2], scalar1=1.0 / GPC)
                gvar = tmpg[:, 2 * cb + 1:2 * cb + 2]
                nc.vector.tensor_mul(out=gvar, in0=gmean, in1=gmean)
                nc.vector.tensor_sub(out=gvar, in0=gm2, in1=gvar)
                # std = sqrt(var + eps)
                nc.scalar.activation(
                    out=gvar, in_=gvar,
                    func=mybir.ActivationFunctionType.Sqrt, bias=eps_t[:, 0:1],
                    scale=1.0)
                nc.vector.reciprocal(out=gvals[:, 2 * cb:2 * cb + 1], in_=gvar)

            # ---- broadcast to per-channel: sm cols [s0, m0, s1, m1] ----
            bc = ps_small.tile([P, 4], F32, tag="ps")
            nc.tensor.matmul(bc, lhsT=gmaskT, rhs=gvals, start=True, stop=True)
            sm = smalls.tile([P, 4], F32, tag="sm")
            nc.vector.tensor_copy(out=sm, in_=bc)

            # ---- per-cb small prep ----
            vss, diags, augs = [], [], []
            kvec = ps_small.tile([R, 1], F32, tag="ps")
            for cb in range(CB):
                s_ap = sm[:, 2 * cb:2 * cb + 1]
                m_ap = sm[:, 2 * cb + 1:2 * cb + 2]
                vt = smalls.tile([P, R], F32, tag=f"vt{cb}")
                nc.sync.dma_start(out=vt, in_=v_d[b, cb])
                # f32 copy for the (tiny, f32) kvec matmul
                vsf = smalls.tile([P, R], F32, tag=f"vsf{cb}")
                nc.vector.tensor_scalar_mul(out=vsf, in0=vt, scalar1=s_ap)
                # bf16 copy as stage-A lhsT
                vs = smalls.tile([P, R], BF16, tag=f"vs{cb}")
                nc.vector.tensor_copy(out=vs, in_=vsf)
                diag = smalls.tile([P, P], BF16, tag=f"diag{cb}")
                nc.vector.tensor_scalar_mul(out=diag, in0=ident_bf, scalar1=s_ap)
                nc.tensor.matmul(
                    kvec, lhsT=vsf, rhs=m_ap, start=(cb == 0), stop=(cb == CB - 1))
                vss.append(vs)
                diags.append(diag)
            kvs = smalls.tile([R, 1], F32, tag="kvs")
            nc.vector.tensor_copy(out=kvs, in_=kvec)

            for cb in range(CB):
                s_ap = sm[:, 2 * cb:2 * cb + 1]
                m_ap = sm[:, 2 * cb + 1:2 * cb + 2]
                # f32 u^T for the tiny ukv matmul comes from casting the
                # bf16 input (exactness is irrelevant: u IS bf16 on stage B)
                aug = smalls.tile([R + 1, P], BF16, tag=f"aug{cb}")
                nc.sync.dma_start(out=aug[0:R, :], in_=ut_d[b, cb])
                utf = smalls.tile([R, P], F32, tag=f"utf{cb}")
                nc.vector.tensor_copy(out=utf, in_=aug[0:R, :])
                ukv = ps_small.tile([P, 1], F32, tag="ps")
                nc.tensor.matmul(ukv, lhsT=utf, rhs=kvs, start=True, stop=True)
                shf = smalls.tile([P, 1], F32, tag=f"shf{cb}")
                nc.sync.dma_start(out=shf, in_=shift_d[b, cb])
                cst = smalls.tile([P, 1], F32, tag=f"cst{cb}")
                nc.vector.tensor_mul(out=cst, in0=m_ap, in1=s_ap)
                nc.vector.tensor_sub(out=cst, in0=shf, in1=cst)
                nc.vector.tensor_sub(out=cst, in0=cst, in1=ukv)
                ctp = ps_small.tile([1, P], F32, tag="ps")
                nc.tensor.transpose(out=ctp, in_=cst, identity=ident)
                cstrow = smalls.tile([1, P], BF16, tag=f"cstrow{cb}")
                nc.scalar.copy(out=cstrow, in_=ctp)
                # compute engines can't write at start partition 12; DMA can
                nc.sync.dma_start(out=aug[R:R + 1, :], in_=cstrow)
                augs.append(aug)

            # ---- stage A: vtx[r, n] (bf16 in, f32 psum) ----
            vaug = vaugs[b % 2]
            for j in range(NCK):
                vtxp = ps_vtx.tile([R, 512], F32, tag="vtx")
                for cb in range(CB):
                    nc.tensor.matmul(
                        vtxp,
                        lhsT=vss[cb],
                        rhs=xbt[(cb, j // 4)][:, (j % 4) * 512:(j % 4 + 1) * 512],
                        start=(cb == 0), stop=(cb == CB - 1))
                nc.scalar.copy(out=vaug[0:R, j * 512:(j + 1) * 512], in_=vtxp)

            # ---- stage B: out = s*x + u@vtx + const ----
            for cb in range(CB):
                for h in range(2):
                    osb = outp.tile([P, 2048], F32, tag="osb")
                    for q in range(4):
                        j = h * 4 + q
                        o_ap = osb[:, q * 512:(q + 1) * 512]
                        pm = ps_out.tile([P, 512], F32, tag="pout")
                        use_stt = stt_mod > 0 and (j + cb) % stt_mod == 0
                        nc.tensor.matmul(
                            pm, lhsT=augs[cb],
                            rhs=vaug[:, j * 512:(j + 1) * 512],
                            start=True, stop=use_stt)
                        if use_stt:
                            nc.vector.scalar_tensor_tensor(
                                out=o_ap,
                                in0=xt[(cb, h)][:, q * 512:(q + 1) * 512],
                                scalar=sm[:, 2 * cb:2 * cb + 1], in1=pm,
                                op0=_MULT, op1=_ADD)
                        else:
                            nc.tensor.matmul(
                                pm, lhsT=diags[cb],
                                rhs=xbt[(cb, h)][:, q * 512:(q + 1) * 512],
                                start=False, stop=True)
                            nc.scalar.copy(out=o_ap, in_=pm)
                    nc.sync.dma_start(
                        out=out_d[b, cb * P:(cb + 1) * P, h * 2048:(h + 1) * 2048],
                        in_=osb)
    # Bacc defers register allocation to its compile()/finalize() pipeline
    nc.finalize()
    return nc


def _host_prep(x, ccm_params):
    import ml_dtypes

    x = np.ascontiguousarray(np.asarray(x, dtype=np.float32).reshape(B, C, HW))
    cp = np.asarray(ccm_params, dtype=np.float32)
    u = cp[:, :C * R].reshape(B, C, R)
    v = cp[:, C * R:2 * C * R].reshape(B, C, R)
    shift = cp[:, 2 * C * R:].reshape(B, C)
    ut = np.ascontiguousarray(
        u.reshape(B, CB, P, R).transpose(0, 1, 3, 2)).astype(np.float16)
    vv = np.ascontiguousarray(v.reshape(B, CB, P, R))
    sh = np.ascontiguousarray(shift.reshape(B, CB, P, 1))
    gmask = np.zeros((P, 16), np.float32)
    gmask[np.arange(P), np.arange(P) // GPC] = 1.0
    gmaskT = np.ascontiguousarray(gmask.T)
    in_maps = []
    for c in range(N_CORES):
        bs = slice(c * BPC, (c + 1) * BPC)
        in_maps.append({
            "x": x[bs], "ut": ut[bs], "v": vv[bs], "shift": sh[bs],
            "gmask": gmask, "gmaskT": gmaskT,
        })
    return in_maps


def kernel(x, ccm_params, _trace=False, _stt_mod=2):
    in_maps = _host_prep(x, ccm_params)
    nc = build_nc(stt_mod=_stt_mod)
    res = run_bass_kernel_spmd(
        nc, in_maps, core_ids=list(range(N_CORES)), trace=_trace)
    out = np.concatenate([r["out"] for r in res.results], axis=0)
    out = out.reshape(B, C, H, W)
    if _trace:
        return out, res
    return out


# revision 11
# speedup vs baseline: 2.0577x; 1.6194x over previous
"""CCMLite kernel for Trainium2: GroupNorm(affine=False) + low-rank channel mix.

out = x_norm + u @ (v^T @ x_norm) + shift, with x_norm = groupnorm(x).

Sharding: data-parallel over batch B=16 across 8 cores (2 batch elems/core).
No collectives needed.

x is shipped to the device as fp16 (host-side cast): GroupNorm statistics
from fp16 inputs are accurate to ~1e-5 relative (population stats over 32k
elements average out the rounding), the matmul datapath wants fp16 anyway
(fp32 matmuls cost 2 PE passes), and it halves the HBM read traffic —
per-core HBM becomes 4.2 MB in + 8.4 MB out (f32).

Device-side algebra (per batch element):
  per-channel stats (bn_stats/bn_aggr on DVE, fp16 in / f32 stats)
  group stats via mask matmul (PE)       -> group mean m_g, rstd s_g
  broadcast to channels via maskT matmul (PE)
  vs[c,r]  = v[c,r] * s_c
  kvec[r]  = sum_c vs[c,r] * m_c
  const_c  = shift_c - m_c*s_c - sum_r u[c,r]*kvec[r]
  vtx[r,n] = sum_c vs[c,r] * x[c,n]      (PE, fp16, K=128 x2)
  out[c,n] = s_c*x[c,n] + sum_r u[c,r]*vtx[r,n] + const_c
 where the last line is computed either as
   (a) PE: aug-matmul (K=13: rows u^T + const row against [vtx; ones])
       + diag(rstd) matmul accumulate (fp16), then ACT copies PSUM->SBUF, or
   (b) PE aug-matmul only, then DVE scalar_tensor_tensor:
       out = (x * s_c) + psum_mixed
 chunks alternate (a)/(b) to balance ACT and DVE load.
"""

from contextlib import ExitStack

import numpy as np

import concourse.bass as bass
import concourse.tile as tile
from concourse import bacc, mybir
from concourse.bass_utils import run_bass_kernel_spmd
from concourse.masks import make_identity

N_CORES = 8
B, C, H, W = 16, 256, 64, 64
HW = H * W            # 4096
R = 12                # low rank
G = 32                # groups
GPC = C // G          # 8 channels per group
P = 128               # partitions
CB = C // P           # 2 channel blocks
BPC = B // N_CORES    # 2 batch elements per core
NCK = HW // 512       # 8 chunks of 512
EPS = 1e-6
F32 = mybir.dt.float32
F16 = mybir.dt.float16

_MULT = mybir.AluOpType.mult
_ADD = mybir.AluOpType.add


def build_nc(stt_mod=2):
    """Build the per-core Bass program. stt_mod: chunk j uses the DVE
    scalar_tensor_tensor path when (j + cb) % stt_mod == 0, else the
    PE-diag + ACT-copy path. stt_mod=0 -> always PE path."""
    nc = bacc.Bacc(None, target_bir_lowering=False)
    x_d = nc.dram_tensor("x", [BPC, C, HW], F16, kind="ExternalInput")
    ut_d = nc.dram_tensor("ut", [BPC, CB, R, P], F16, kind="ExternalInput")
    v_d = nc.dram_tensor("v", [BPC, CB, P, R], F32, kind="ExternalInput")
    shift_d = nc.dram_tensor("shift", [BPC, CB, P, 1], F32, kind="ExternalInput")
    gmask_d = nc.dram_tensor("gmask", [P, 16], F32, kind="ExternalInput")
    gmaskT_d = nc.dram_tensor("gmaskT", [16, P], F32, kind="ExternalInput")
    out_d = nc.dram_tensor("out", [BPC, C, HW], F32, kind="ExternalOutput")

    with tile.TileContext(nc) as tc, ExitStack() as ctx:
        consts = ctx.enter_context(tc.tile_pool(name="consts", bufs=1))
        xbp = ctx.enter_context(tc.tile_pool(name="xbp", bufs=8))
        outp = ctx.enter_context(tc.tile_pool(name="outp", bufs=6))
        smalls = ctx.enter_context(tc.tile_pool(name="smalls", bufs=4))
        ps_small = ctx.enter_context(tc.tile_pool(name="ps_small", bufs=2, space="PSUM"))
        ps_vtx = ctx.enter_context(tc.tile_pool(name="ps_vtx", bufs=2, space="PSUM"))
        ps_out = ctx.enter_context(tc.tile_pool(name="ps_out", bufs=3, space="PSUM"))

        ident = consts.tile([P, P], F32)
        make_identity(nc, ident)
        ident_h = consts.tile([P, P], F16)
        make_identity(nc, ident_h)
        gmask = consts.tile([P, 16], F32)
        nc.sync.dma_start(out=gmask, in_=gmask_d[:, :])
        gmaskT = consts.tile([16, P], F32)
        nc.sync.dma_start(out=gmaskT, in_=gmaskT_d[:, :])
        eps_t = consts.tile([16, 1], F32)
        nc.vector.memset(eps_t, EPS)
        # two persistent vtx-aug rhs tiles (rows 0..R-1 rewritten per batch,
        # row R stays 1.0); memset hoisted off the per-batch path
        vaugs = []
        for i in range(2):
            va = consts.tile([R + 1, HW], F16, tag=f"vaug{i}")
            nc.gpsimd.memset(va, 1.0)
            vaugs.append(va)

        for b in range(BPC):
            # ---- load x (fp16): 4 tiles of [128, 2048] ----
            xbt = {}
            for cb in range(CB):
                for h in range(2):
                    tb = xbp.tile([P, 2048], F16, tag="xbt")
                    nc.sync.dma_start(
                        out=tb,
                        in_=x_d[b, cb * P:(cb + 1) * P, h * 2048:(h + 1) * 2048],
                    )
                    xbt[(cb, h)] = tb

            # ---- per-channel stats (fp16 in, f32 stats) ----
            mvs = []
            for cb in range(CB):
                st = smalls.tile([P, NCK, 6], F32, tag="bstats")
                for j in range(NCK):
                    nc.vector.bn_stats(
                        out=st[:, j:j + 1, :],
                        in_=xbt[(cb, j // 4)][:, (j % 4) * 512:(j % 4 + 1) * 512],
                    )
                mv = smalls.tile([P, 2], F32, tag=f"mv{cb}")
                nc.vector.bn_aggr(out=mv, in_=st)
                mvs.append(mv)

            # chs cols: [mean_cb0, E2_cb0, mean_cb1, E2_cb1]
            chs = smalls.tile([P, 4], F32, tag="chs")
            for cb in range(CB):
                mv = mvs[cb]
                nc.vector.tensor_copy(out=chs[:, 2 * cb:2 * cb + 1], in_=mv[:, 0:1])
                nc.vector.tensor_mul(
                    out=chs[:, 2 * cb + 1:2 * cb + 2], in0=mv[:, 0:1], in1=mv[:, 0:1])
                nc.vector.tensor_add(
                    out=chs[:, 2 * cb + 1:2 * cb + 2],
                    in0=chs[:, 2 * cb + 1:2 * cb + 2], in1=mv[:, 1:2])

            # ---- group stats: gsum[g, :] = sum over 8 chans ----
            gsum = ps_small.tile([16, 4], F32, tag="ps")
            nc.tensor.matmul(gsum, lhsT=gmask, rhs=chs, start=True, stop=True)

            # gvals cols: [rstd_cb0, mean_cb0, rstd_cb1, mean_cb1]
            gvals = smalls.tile([16, 4], F32, tag="gvals")
            tmpg = smalls.tile([16, 4], F32, tag="tmpg")
            for cb in range(CB):
                gmean = gvals[:, 2 * cb + 1:2 * cb + 2]
                nc.vector.tensor_scalar_mul(
                    out=gmean, in0=gsum[:, 2 * cb:2 * cb + 1], scalar1=1.0 / GPC)
                gm2 = tmpg[:, 2 * cb:2 * cb + 1]
                nc.vector.tensor_scalar_mul(
                    out=gm2, in0=gsum[:, 2 * cb + 1:2 * cb + 2], scalar1=1.0 / GPC)
                gvar = tmpg[:, 2 * cb + 1:2 * cb + 2]
                nc.vector.tensor_mul(out=gvar, in0=gmean, in1=gmean)
                nc.vector.tensor_sub(out=gvar, in0=gm2, in1=gvar)
                # std = sqrt(var + eps)
                nc.scalar.activation(
                    out=gvar, in_=gvar,
                    func=mybir.ActivationFunctionType.Sqrt, bias=eps_t[:, 0:1],
                    scale=1.0)
                nc.vector.reciprocal(out=gvals[:, 2 * cb:2 * cb + 1], in_=gvar)

            # ---- broadcast to per-channel: sm cols [s0, m0, s1, m1] ----
            bc = ps_small.tile([P, 4], F32, tag="ps")
            nc.tensor.matmul(bc, lhsT=gmaskT, rhs=gvals, start=True, stop=True)
            sm = smalls.tile([P, 4], F32, tag="sm")
            nc.vector.tensor_copy(out=sm, in_=bc)

            # ---- per-cb small prep ----
            vss, diags, augs = [], [], []
            kvec = ps_small.tile([R, 1], F32, tag="ps")
            for cb in range(CB):
                s_ap = sm[:, 2 * cb:2 * cb + 1]
                vt = smalls.tile([P, R], F32, tag=f"vt{cb}")
                nc.sync.dma_start(out=vt, in_=v_d[b, cb])
                # f32 copy for the (tiny, f32) kvec matmul
                vsf = smalls.tile([P, R], F32, tag=f"vsf{cb}")
                nc.vector.tensor_scalar_mul(out=vsf, in0=vt, scalar1=s_ap)
                # fp16 copy as stage-A lhsT
                vs = smalls.tile([P, R], F16, tag=f"vs{cb}")
                nc.vector.tensor_copy(out=vs, in_=vsf)
                diag = smalls.tile([P, P], F16, tag=f"diag{cb}")
                nc.vector.tensor_scalar_mul(out=diag, in0=ident_h, scalar1=s_ap)
                nc.tensor.matmul(
                    kvec, lhsT=vsf, rhs=sm[:, 2 * cb + 1:2 * cb + 2],
                    start=(cb == 0), stop=(cb == CB - 1))
                vss.append(vs)
                diags.append(diag)
            kvs = smalls.tile([R, 1], F16, tag="kvs")
            nc.vector.tensor_copy(out=kvs, in_=kvec)

            for cb in range(CB):
                s_ap = sm[:, 2 * cb:2 * cb + 1]
                m_ap = sm[:, 2 * cb + 1:2 * cb + 2]
                aug = smalls.tile([R + 1, P], F16, tag=f"aug{cb}")
                nc.sync.dma_start(out=aug[0:R, :], in_=ut_d[b, cb])
                ukv = ps_small.tile([P, 1], F32, tag="ps")
                nc.tensor.matmul(ukv, lhsT=aug[0:R, :], rhs=kvs, start=True,
                                 stop=True)
                shf = smalls.tile([P, 1], F32, tag=f"shf{cb}")
                nc.sync.dma_start(out=shf, in_=shift_d[b, cb])
                cst = smalls.tile([P, 1], F32, tag=f"cst{cb}")
                nc.vector.tensor_mul(out=cst, in0=m_ap, in1=s_ap)
                nc.vector.tensor_sub(out=cst, in0=shf, in1=cst)
                nc.vector.tensor_sub(out=cst, in0=cst, in1=ukv)
                ctp = ps_small.tile([1, P], F32, tag="ps")
                nc.tensor.transpose(out=ctp, in_=cst, identity=ident)
                cstrow = smalls.tile([1, P], F16, tag=f"cstrow{cb}")
                nc.scalar.copy(out=cstrow, in_=ctp)
                # compute engines can't write at start partition 12; DMA can
                nc.sync.dma_start(out=aug[R:R + 1, :], in_=cstrow)
                augs.append(aug)

            # ---- stage A: vtx[r, n] (fp16 in, f32 psum) ----
            vaug = vaugs[b % 2]
            for j in range(NCK):
                vtxp = ps_vtx.tile([R, 512], F32, tag="vtx")
                for cb in range(CB):
                    nc.tensor.matmul(
                        vtxp,
                        lhsT=vss[cb],
                        rhs=xbt[(cb, j // 4)][:, (j % 4) * 512:(j % 4 + 1) * 512],
                        start=(cb == 0), stop=(cb == CB - 1))
                nc.scalar.copy(out=vaug[0:R, j * 512:(j + 1) * 512], in_=vtxp)

            # ---- stage B: out = s*x + u@vtx + const ----
            for cb in range(CB):
                for h in range(2):
                    osb = outp.tile([P, 2048], F32, tag="osb")
                    for q in range(4):
                        j = h * 4 + q
                        o_ap = osb[:, q * 512:(q + 1) * 512]
                        pm = ps_out.tile([P, 512], F32, tag="pout")
                        use_stt = stt_mod > 0 and (j + cb) % stt_mod == 0
                        nc.tensor.matmul(
                            pm, lhsT=augs[cb],
                            rhs=vaug[:, j * 512:(j + 1) * 512],
                            start=True, stop=use_stt)
                        if use_stt:
                            nc.vector.scalar_tensor_tensor(
                                out=o_ap,
                                in0=xbt[(cb, h)][:, q * 512:(q + 1) * 512],
                                scalar=sm[:, 2 * cb:2 * cb + 1], in1=pm,
                                op0=_MULT, op1=_ADD)
                        else:
                            nc.tensor.matmul(
                                pm, lhsT=diags[cb],
                                rhs=xbt[(cb, h)][:, q * 512:(q + 1) * 512],
                                start=False, stop=True)
                            nc.scalar.copy(out=o_ap, in_=pm)
                    nc.sync.dma_start(
                        out=out_d[b, cb * P:(cb + 1) * P, h * 2048:(h + 1) * 2048],
                        in_=osb)
    # Bacc defers register allocation to its compile()/finalize() pipeline
    nc.finalize()
    return nc


def _host_prep(x, ccm_params):
    x = np.asarray(x, dtype=np.float32).reshape(B, C, HW).astype(np.float16)
    x = np.ascontiguousarray(x)
    cp = np.asarray(ccm_params, dtype=np.float32)
    u = cp[:, :C * R].reshape(B, C, R)
    v = cp[:, C * R:2 * C * R].reshape(B, C, R)
    shift = cp[:, 2 * C * R:].reshape(B, C)
    ut = np.ascontiguousarray(
        u.reshape(B, CB, P, R).transpose(0, 1, 3, 2)).astype(np.float16)
    vv = np.ascontiguousarray(v.reshape(B, CB, P, R))
    sh = np.ascontiguousarray(shift.reshape(B, CB, P, 1))
    gmask = np.zeros((P, 16), np.float32)
    gmask[np.arange(P), np.arange(P) // GPC] = 1.0
    gmaskT = np.ascontiguousarray(gmask.T)
    in_maps = []
    for c in range(N_CORES):
        bs = slice(c * BPC, (c + 1) * BPC)
        in_maps.append({
            "x": x[bs], "ut": ut[bs], "v": vv[bs], "shift": sh[bs],
            "gmask": gmask, "gmaskT": gmaskT,
        })
    return in_maps


def kernel(x, ccm_params, _trace=False, _stt_mod=2):
    in_maps = _host_prep(x, ccm_params)
    nc = build_nc(stt_mod=_stt_mod)
    res = run_bass_kernel_spmd(
        nc, in_maps, core_ids=list(range(N_CORES)), trace=_trace)
    out = np.concatenate([r["out"] for r in res.results], axis=0)
    out = out.reshape(B, C, H, W)
    if _trace:
        return out, res
    return out
